# revision 5
# baseline (speedup 1.0000x reference)
"""CCALayer3D kernel for 8 Trainium2 NeuronCores.

reference semantics (x: [4, 64, 32, 128, 128] f32):
    mean/var over (D,H,W) per (B,C); y = std + mean
    h = relu(w1 @ y + b1); g = sigmoid(w2 @ h + b2)
    out = x * g[:, :, None, None, None]

Sharding: core i handles batch b = i//2, D-half t = i%2 (16 of 32 d-slices,
64 MiB per core).  Per-core layout [128, 131072]: partition p = s*64 + c where
s splits the core's 16 d-slices into two groups of 8 (so all 128 partitions
carry DMA traffic).  Per-channel sum/sumsq partials are computed on-device
(DVE reduce + ACT square-accumulate, one pass each), exchanged between the two
cores of a batch with a tiny pair AllGather, the MLP runs redundantly on every
core, and a second pass rescales x by g.  Tile width 16384 keeps DMA
descriptors at 64 KiB (measured 384 GB/s read, vs 315 at 32 KiB).
"""

import numpy as np

_B, _C = 4, 64
_HW = 128 * 128
_FREE = 8 * _HW            # 131072 free elems per partition
_TILE_N = 16384
_NT = _FREE // _TILE_N     # 8 tiles per pass
_NRED = 32 * _HW           # 524288 elements reduced per (b, c)
_NCORES = 8

# test-harness knobs (the grading harness just calls kernel())
TRACE = False
TRACE_KWARGS = {}
LAST_RESULT = None

_cached_nc = None


def _build():
    import concourse.bacc as bacc
    import concourse.tile as tile
    from concourse import mybir

    nc = bacc.Bacc("TRN2", target_bir_lowering=False, debug=False,
                   num_devices=_NCORES)
    f32 = mybir.dt.float32
    AX = mybir.AxisListType.X
    AF = mybir.ActivationFunctionType

    x = nc.dram_tensor("x", [128, _FREE], f32, kind="ExternalInput")
    out = nc.dram_tensor("out", [128, _FREE], f32, kind="ExternalOutput")
    w1t = nc.dram_tensor("w1t", [64, 4], f32, kind="ExternalInput")
    b1 = nc.dram_tensor("b1", [4, 1], f32, kind="ExternalInput")
    w2t = nc.dram_tensor("w2t", [4, 128], f32, kind="ExternalInput")
    b2 = nc.dram_tensor("b2", [128, 1], f32, kind="ExternalInput")

    with tile.TileContext(nc) as tc:
        with (
            tc.tile_pool(name="xp", bufs=2) as xp,
            tc.tile_pool(name="small", bufs=1) as small,
            tc.tile_pool(name="psum", bufs=2, space="PSUM") as psum,
            tc.tile_pool(name="dram", bufs=1, space="DRAM") as dram,
        ):
            # MLP weights prefetched up front; overlap with pass 1
            w1t_sb = small.tile([64, 4], f32)
            nc.gpsimd.dma_start(w1t_sb[:], w1t[:])
            b1_sb = small.tile([4, 1], f32)
            nc.gpsimd.dma_start(b1_sb[:], b1[:])
            w2t_sb = small.tile([4, 128], f32)
            nc.gpsimd.dma_start(w2t_sb[:], w2t[:])
            b2_sb = small.tile([128, 1], f32)
            nc.gpsimd.dma_start(b2_sb[:], b2[:])

            # ---- pass 1: per-partition mean/M2 via bn_stats (one DVE read) --
            ngrp = _TILE_N // 512           # bn_stats groups per tile
            bns = small.tile([128, _NT * ngrp * 6], f32)
            for j in range(_NT):
                t = xp.tile([128, _TILE_N], f32, tag="xtile")
                nc.sync.dma_start(t[:], x[:, j * _TILE_N:(j + 1) * _TILE_N])
                for k in range(ngrp):
                    gidx = j * ngrp + k
                    nc.vector.bn_stats(bns[:, gidx * 6:(gidx + 1) * 6],
                                       t[:, k * 512:(k + 1) * 512])

            # aggregate all groups -> per-partition (mean, var)
            mv = small.tile([128, 2], f32)
            nc.vector.bn_aggr(mv[:], bns[:].rearrange("p (g k) -> p g k", k=6))
            # convert to (sum, sumsq) so partials add across partitions/cores
            msq_p = small.tile([128, 1], f32)
            nc.vector.tensor_mul(msq_p[:], mv[:, 0:1], mv[:, 0:1])
            e2_p = small.tile([128, 1], f32)
            nc.vector.tensor_add(e2_p[:], mv[:, 1:2], msq_p[:])
            stats = small.tile([128, 2], f32)
            nc.vector.tensor_scalar_mul(stats[:, 0:1], mv[:, 0:1], float(_FREE))
            nc.vector.tensor_scalar_mul(stats[:, 1:2], e2_p[:], float(_FREE))

            # ---- pair AllGather of partial stats (cheaper than AllReduce) --
            cc_in = dram.tile([128, 2], f32)
            cc_out = dram.tile([256, 2], f32)
            nc.gpsimd.dma_start(cc_in[:], stats[:])
            nc.gpsimd.collective_compute(
                "AllGather", mybir.AluOpType.bypass,
                replica_groups=[[0, 1], [2, 3], [4, 5], [6, 7]],
                ins=[cc_in.opt()], outs=[cc_out.opt()],
            )
            # cc_out rows: [own s0 | own s1 | peer s0 | peer s1] x 64 channels.
            # Fetch as [c, k, g] and reduce over the 4 groups.
            part4 = small.tile([64, 2, 4], f32)
            nc.gpsimd.dma_start(part4[:],
                                cc_out[:].rearrange("(g c) k -> c k g", c=64))
            tot = small.tile([64, 2], f32)
            nc.vector.reduce_sum(tot[:], part4[:], axis=AX)

            # ---- moments -> y = std + mean ----
            mom = small.tile([64, 2], f32)           # [mean, E[x^2]]
            nc.vector.tensor_scalar_mul(mom[:], tot[:], 1.0 / _NRED)
            msq = small.tile([64, 1], f32)
            nc.vector.tensor_mul(msq[:], mom[:, 0:1], mom[:, 0:1])
            var = small.tile([64, 1], f32)
            nc.vector.tensor_sub(var[:], mom[:, 1:2], msq[:])
            std = small.tile([64, 1], f32)
            nc.scalar.activation(std[:], var[:], AF.Sqrt)
            y = small.tile([64, 1], f32)
            nc.vector.tensor_add(y[:], std[:], mom[:, 0:1])

            # ---- MLP: h = relu(w1 @ y + b1); g = sigmoid(w2 @ h + b2) ----
            ph = psum.tile([4, 1], f32)
            nc.tensor.matmul(ph[:], w1t_sb[:], y[:])
            h = small.tile([4, 1], f32)
            nc.scalar.activation(h[:], ph[:], AF.Relu, bias=b1_sb[:, 0:1])
            # w2t is [w2.T | w2.T] so the matmul emits g duplicated over both
            # partition halves, matching the x layout
            pg = psum.tile([128, 1], f32)
            nc.tensor.matmul(pg[:], w2t_sb[:], h[:])
            g = small.tile([128, 1], f32)
            nc.scalar.activation(g[:], pg[:], AF.Sigmoid, bias=b2_sb[:, 0:1])

            # ---- pass 2: out = x * g (in-place DVE mult, stores on ACT ring)
            for j in range(_NT):
                t = xp.tile([128, _TILE_N], f32, tag="xtile")
                nc.sync.dma_start(t[:], x[:, j * _TILE_N:(j + 1) * _TILE_N])
                nc.vector.tensor_scalar_mul(t[:], t[:], g[:, 0:1])
                nc.scalar.dma_start(out[:, j * _TILE_N:(j + 1) * _TILE_N], t[:])

    nc.compile()
    return nc


def kernel(x, w1, b1, w2, b2):
    global _cached_nc, LAST_RESULT
    from concourse.bass_utils import run_bass_kernel_spmd

    x = np.ascontiguousarray(np.asarray(x, dtype=np.float32))
    w1 = np.asarray(w1, dtype=np.float32)
    b1 = np.asarray(b1, dtype=np.float32)
    w2 = np.asarray(w2, dtype=np.float32)
    b2 = np.asarray(b2, dtype=np.float32)

    if _cached_nc is None:
        _cached_nc = _build()
    nc = _cached_nc

    w1t = np.ascontiguousarray(w1.T)                                  # [64, 4]
    b1c = np.ascontiguousarray(b1.reshape(4, 1))
    w2t = np.ascontiguousarray(np.concatenate([w2.T, w2.T], axis=1))  # [4, 128]
    b2c = np.ascontiguousarray(np.concatenate([b2, b2]).reshape(128, 1))

    # x[b, c, d, h, w] -> [b, c, q, n] with q = d//8, n over 8 d-slices
    xv = x.reshape(_B, _C, 4, _FREE)
    in_maps = []
    for i in range(_NCORES):
        b, t = divmod(i, 2)
        xs = np.empty((2, _C, _FREE), np.float32)
        xs[0] = xv[b, :, 2 * t]
        xs[1] = xv[b, :, 2 * t + 1]
        in_maps.append({
            "x": xs.reshape(128, _FREE),
            "w1t": w1t, "b1": b1c, "w2t": w2t, "b2": b2c,
        })

    res = run_bass_kernel_spmd(nc, in_maps, list(range(_NCORES)),
                               trace=TRACE, **TRACE_KWARGS)
    LAST_RESULT = res

    outf = np.empty_like(x)
    ov = outf.reshape(_B, _C, 4, _FREE)
    for i in range(_NCORES):
        b, t = divmod(i, 2)
        r = res.results[i]["out"].reshape(2, _C, _FREE)
        ov[b, :, 2 * t] = r[0]
        ov[b, :, 2 * t + 1] = r[1]
    return outf


# revision 8
# speedup vs baseline: 1.1229x; 1.1229x over previous
"""CCALayer3D kernel for 8 Trainium2 NeuronCores.

reference semantics (x: [4, 64, 32, 128, 128] f32):
    mean/var over (D,H,W) per (B,C); y = std + mean
    h = relu(w1 @ y + b1); g = sigmoid(w2 @ h + b2)
    out = x * g[:, :, None, None, None]

Sharding: core i handles batch b = i//2, D-half t = i%2 (16 of 32 d-slices,
64 MiB per core).  Per-core layout [128, 131072]: partition p = s*64 + c where
s splits the core's 16 d-slices into two groups of 8 (so all 128 partitions
carry DMA traffic).  Per-channel sum/sumsq partials are computed on-device
(DVE reduce + ACT square-accumulate, one pass each), exchanged between the two
cores of a batch with a tiny pair AllGather, the MLP runs redundantly on every
core, and a second pass rescales x by g.  Tile width 16384 keeps DMA
descriptors at 64 KiB (measured 384 GB/s read, vs 315 at 32 KiB).
"""

import numpy as np

_B, _C = 4, 64
_HW = 128 * 128
_FREE = 8 * _HW            # 131072 free elems per partition
_TILE_N = 16384
_NT = _FREE // _TILE_N     # 8 tiles per pass
_NRED = 32 * _HW           # 524288 elements reduced per (b, c)
_NCORES = 8

# test-harness knobs (the grading harness just calls kernel())
TRACE = False
TRACE_KWARGS = {}
LAST_RESULT = None

_cached_nc = None


def _build():
    import concourse.bacc as bacc
    import concourse.tile as tile
    from concourse import mybir

    nc = bacc.Bacc("TRN2", target_bir_lowering=False, debug=False,
                   num_devices=_NCORES)
    f32 = mybir.dt.float32
    AX = mybir.AxisListType.X
    AF = mybir.ActivationFunctionType

    x = nc.dram_tensor("x", [128, _FREE], f32, kind="ExternalInput")
    out = nc.dram_tensor("out", [128, _FREE], f32, kind="ExternalOutput")
    w1t = nc.dram_tensor("w1t", [64, 4], f32, kind="ExternalInput")
    b1 = nc.dram_tensor("b1", [4, 1], f32, kind="ExternalInput")
    w2t = nc.dram_tensor("w2t", [4, 128], f32, kind="ExternalInput")
    b2 = nc.dram_tensor("b2", [128, 1], f32, kind="ExternalInput")

    with tile.TileContext(nc) as tc:
        with (
            tc.tile_pool(name="xp", bufs=3) as xp,
            tc.tile_pool(name="bnp", bufs=2) as bnp,
            tc.tile_pool(name="small", bufs=1) as small,
            tc.tile_pool(name="psum", bufs=2, space="PSUM") as psum,
            tc.tile_pool(name="dram", bufs=1, space="DRAM") as dram,
        ):
            # MLP weights prefetched up front; overlap with pass 1
            w1t_sb = small.tile([64, 4], f32)
            nc.gpsimd.dma_start(w1t_sb[:], w1t[:])
            b1_sb = small.tile([4, 1], f32)
            nc.gpsimd.dma_start(b1_sb[:], b1[:])
            w2t_sb = small.tile([4, 128], f32)
            nc.gpsimd.dma_start(w2t_sb[:], w2t[:])
            b2_sb = small.tile([128, 1], f32)
            nc.gpsimd.dma_start(b2_sb[:], b2[:])

            # ---- pass 1: per-partition mean/M2 via bn_stats (one DVE read) --
            ngrp = _TILE_N // 512           # bn_stats groups per tile
            mv_cols = small.tile([128, 2, _NT], f32)   # (mean, var) per tile
            for j in range(_NT):
                t = xp.tile([128, _TILE_N], f32, tag="xtile")
                nc.sync.dma_start(t[:], x[:, j * _TILE_N:(j + 1) * _TILE_N])
                bnst = bnp.tile([128, ngrp * 6], f32, tag="bnst")
                for k in range(ngrp):
                    nc.vector.bn_stats(bnst[:, k * 6:(k + 1) * 6],
                                       t[:, k * 512:(k + 1) * 512])
                nc.vector.bn_aggr(mv_cols[:, :, j:j + 1],
                                  bnst[:].rearrange("p (g k) -> p g k", k=6))

            # merge the _NT per-tile (mean, var) pairs (equal counts):
            # mean = avg(mean_j), E2 = avg(var_j + mean_j^2)
            means = mv_cols[:, 0, :]
            varis = mv_cols[:, 1, :]
            msq8 = small.tile([128, _NT], f32)
            nc.vector.tensor_mul(msq8[:], means, means)
            e28 = small.tile([128, _NT], f32)
            nc.vector.tensor_add(e28[:], varis, msq8[:])
            # stats carries (sum(mean_j), sum(e2_j)); scales fold into 1/32
            stats = small.tile([128, 2], f32)
            nc.vector.reduce_sum(stats[:, 0:1], means, axis=AX)
            nc.vector.reduce_sum(stats[:, 1:2], e28[:], axis=AX)

            # ---- pair AllGather of partial stats (cheaper than AllReduce) --
            cc_in = dram.tile([128, 2], f32)
            cc_out = dram.tile([256, 2], f32)
            nc.gpsimd.dma_start(cc_in[:], stats[:])
            nc.gpsimd.collective_compute(
                "AllGather", mybir.AluOpType.bypass,
                replica_groups=[[0, 1], [2, 3], [4, 5], [6, 7]],
                ins=[cc_in.opt()], outs=[cc_out.opt()],
            )
            # cc_out rows: [own s0 | own s1 | peer s0 | peer s1] x 64 channels.
            # Fetch as [c, k, g] and reduce over the 4 groups.
            part4 = small.tile([64, 2, 4], f32)
            nc.gpsimd.dma_start(part4[:],
                                cc_out[:].rearrange("(g c) k -> c k g", c=64))
            tot = small.tile([64, 2], f32)
            nc.vector.reduce_sum(tot[:], part4[:], axis=AX)

            # ---- moments -> y = std + mean ----
            # tot = sum over 4*_NT equal groups of (mean_g, e2_g)
            mom = small.tile([64, 2], f32)           # [mean, E[x^2]]
            nc.vector.tensor_scalar_mul(mom[:], tot[:], 1.0 / (4 * _NT))
            msq = small.tile([64, 1], f32)
            nc.vector.tensor_mul(msq[:], mom[:, 0:1], mom[:, 0:1])
            var = small.tile([64, 1], f32)
            nc.vector.tensor_sub(var[:], mom[:, 1:2], msq[:])
            std = small.tile([64, 1], f32)
            nc.scalar.activation(std[:], var[:], AF.Sqrt)
            y = small.tile([64, 1], f32)
            nc.vector.tensor_add(y[:], std[:], mom[:, 0:1])

            # ---- MLP: h = relu(w1 @ y + b1); g = sigmoid(w2 @ h + b2) ----
            ph = psum.tile([4, 1], f32)
            nc.tensor.matmul(ph[:], w1t_sb[:], y[:])
            h = small.tile([4, 1], f32)
            nc.scalar.activation(h[:], ph[:], AF.Relu, bias=b1_sb[:, 0:1])
            # w2t is [w2.T | w2.T] so the matmul emits g duplicated over both
            # partition halves, matching the x layout
            pg = psum.tile([128, 1], f32)
            nc.tensor.matmul(pg[:], w2t_sb[:], h[:])
            g = small.tile([128, 1], f32)
            nc.scalar.activation(g[:], pg[:], AF.Sigmoid, bias=b2_sb[:, 0:1])

            # ---- pass 2: out = x * g (in-place DVE mult, stores on ACT ring)
            for j in range(_NT):
                t = xp.tile([128, _TILE_N], f32, tag="xtile")
                nc.sync.dma_start(t[:], x[:, j * _TILE_N:(j + 1) * _TILE_N])
                nc.vector.tensor_scalar_mul(t[:], t[:], g[:, 0:1])
                nc.scalar.dma_start(out[:, j * _TILE_N:(j + 1) * _TILE_N], t[:])

    nc.compile()
    return nc


def kernel(x, w1, b1, w2, b2):
    global _cached_nc, LAST_RESULT
    from concourse.bass_utils import run_bass_kernel_spmd

    x = np.ascontiguousarray(np.asarray(x, dtype=np.float32))
    w1 = np.asarray(w1, dtype=np.float32)
    b1 = np.asarray(b1, dtype=np.float32)
    w2 = np.asarray(w2, dtype=np.float32)
    b2 = np.asarray(b2, dtype=np.float32)

    if _cached_nc is None:
        _cached_nc = _build()
    nc = _cached_nc

    w1t = np.ascontiguousarray(w1.T)                                  # [64, 4]
    b1c = np.ascontiguousarray(b1.reshape(4, 1))
    w2t = np.ascontiguousarray(np.concatenate([w2.T, w2.T], axis=1))  # [4, 128]
    b2c = np.ascontiguousarray(np.concatenate([b2, b2]).reshape(128, 1))

    # x[b, c, d, h, w] -> [b, c, q, n] with q = d//8, n over 8 d-slices
    xv = x.reshape(_B, _C, 4, _FREE)
    in_maps = []
    for i in range(_NCORES):
        b, t = divmod(i, 2)
        xs = np.empty((2, _C, _FREE), np.float32)
        xs[0] = xv[b, :, 2 * t]
        xs[1] = xv[b, :, 2 * t + 1]
        in_maps.append({
            "x": xs.reshape(128, _FREE),
            "w1t": w1t, "b1": b1c, "w2t": w2t, "b2": b2c,
        })

    res = run_bass_kernel_spmd(nc, in_maps, list(range(_NCORES)),
                               trace=TRACE, **TRACE_KWARGS)
    LAST_RESULT = res

    outf = np.empty_like(x)
    ov = outf.reshape(_B, _C, 4, _FREE)
    for i in range(_NCORES):
        b, t = divmod(i, 2)
        r = res.results[i]["out"].reshape(2, _C, _FREE)
        ov[b, :, 2 * t] = r[0]
        ov[b, :, 2 * t + 1] = r[1]
    return outf


# revision 9
# speedup vs baseline: 1.1490x; 1.0232x over previous
"""CCALayer3D kernel for 8 Trainium2 NeuronCores.

reference semantics (x: [4, 64, 32, 128, 128] f32):
    mean/var over (D,H,W) per (B,C); y = std + mean
    h = relu(w1 @ y + b1); g = sigmoid(w2 @ h + b2)
    out = x * g[:, :, None, None, None]

Sharding: core i handles batch b = i//2, D-half t = i%2 (16 of 32 d-slices,
64 MiB per core).  Per-core layout [128, 131072]: partition p = s*64 + c where
s splits the core's 16 d-slices into two groups of 8 (so all 128 partitions
carry DMA traffic).  Per-channel sum/sumsq partials are computed on-device
(DVE reduce + ACT square-accumulate, one pass each), exchanged between the two
cores of a batch with a tiny pair AllGather, the MLP runs redundantly on every
core, and a second pass rescales x by g.  Tile width 16384 keeps DMA
descriptors at 64 KiB (measured 384 GB/s read, vs 315 at 32 KiB).
"""

import numpy as np

_B, _C = 4, 64
_HW = 128 * 128
_FREE = 8 * _HW            # 131072 free elems per partition
_TILE_N = 16384
_NT = _FREE // _TILE_N     # 8 tiles per pass
_NRED = 32 * _HW           # 524288 elements reduced per (b, c)
_NCORES = 8

# test-harness knobs (the grading harness just calls kernel())
TRACE = False
TRACE_KWARGS = {}
LAST_RESULT = None

_cached_nc = None


def _build():
    import concourse.bacc as bacc
    import concourse.tile as tile
    from concourse import mybir

    nc = bacc.Bacc("TRN2", target_bir_lowering=False, debug=False,
                   num_devices=_NCORES)
    f32 = mybir.dt.float32
    AX = mybir.AxisListType.X
    AF = mybir.ActivationFunctionType

    x = nc.dram_tensor("x", [128, _FREE], f32, kind="ExternalInput")
    out = nc.dram_tensor("out", [128, _FREE], f32, kind="ExternalOutput")
    w1t = nc.dram_tensor("w1t", [64, 4], f32, kind="ExternalInput")
    b1 = nc.dram_tensor("b1", [4, 1], f32, kind="ExternalInput")
    w2t = nc.dram_tensor("w2t", [4, 128], f32, kind="ExternalInput")
    b2 = nc.dram_tensor("b2", [128, 1], f32, kind="ExternalInput")

    with tile.TileContext(nc) as tc:
        with (
            tc.tile_pool(name="xp", bufs=3) as xp,
            tc.tile_pool(name="bnp", bufs=2) as bnp,
            tc.tile_pool(name="small", bufs=1) as small,
            tc.tile_pool(name="psum", bufs=2, space="PSUM") as psum,
            tc.tile_pool(name="dram", bufs=1, space="DRAM") as dram,
        ):
            # MLP weights prefetched up front; overlap with pass 1
            w1t_sb = small.tile([64, 4], f32)
            nc.gpsimd.dma_start(w1t_sb[:], w1t[:])
            b1_sb = small.tile([4, 1], f32)
            nc.gpsimd.dma_start(b1_sb[:], b1[:])
            w2t_sb = small.tile([4, 128], f32)
            nc.gpsimd.dma_start(w2t_sb[:], w2t[:])
            b2_sb = small.tile([128, 1], f32)
            nc.gpsimd.dma_start(b2_sb[:], b2[:])

            # ---- pass 1: per-partition mean/M2 via bn_stats (one DVE read) --
            ngrp = _TILE_N // 512           # bn_stats groups per tile
            mv_cols = small.tile([128, 2, _NT], f32)   # (mean, var) per tile
            for j in range(_NT):
                t = xp.tile([128, _TILE_N], f32, tag="xtile")
                nc.sync.dma_start(t[:], x[:, j * _TILE_N:(j + 1) * _TILE_N])
                bnst = bnp.tile([128, ngrp * 6], f32, tag="bnst")
                for k in range(ngrp):
                    nc.vector.bn_stats(bnst[:, k * 6:(k + 1) * 6],
                                       t[:, k * 512:(k + 1) * 512])
                nc.vector.bn_aggr(mv_cols[:, :, j:j + 1],
                                  bnst[:].rearrange("p (g k) -> p g k", k=6))

            # merge the _NT per-tile (mean, var) pairs (equal counts):
            # mean = avg(mean_j), E2 = avg(var_j + mean_j^2)
            means = mv_cols[:, 0, :]
            varis = mv_cols[:, 1, :]
            msq8 = small.tile([128, _NT], f32)
            nc.vector.tensor_mul(msq8[:], means, means)
            e28 = small.tile([128, _NT], f32)
            nc.vector.tensor_add(e28[:], varis, msq8[:])
            # stats carries (sum(mean_j), sum(e2_j)); scales fold into 1/32
            stats = small.tile([128, 2], f32)
            nc.vector.reduce_sum(stats[:, 0:1], means, axis=AX)
            nc.vector.reduce_sum(stats[:, 1:2], e28[:], axis=AX)

            # ---- pair AllGather of partial stats (cheaper than AllReduce) --
            cc_in = dram.tile([128, 2], f32)
            cc_out = dram.tile([256, 2], f32)
            nc.gpsimd.dma_start(cc_in[:], stats[:])
            nc.gpsimd.collective_compute(
                "AllGather", mybir.AluOpType.bypass,
                replica_groups=[[0, 1], [2, 3], [4, 5], [6, 7]],
                ins=[cc_in.opt()], outs=[cc_out.opt()],
            )
            # cc_out rows: [own s0 | own s1 | peer s0 | peer s1] x 64 channels.
            # Fetch as [c, k, g] and reduce over the 4 groups.
            part4 = small.tile([64, 2, 4], f32)
            nc.gpsimd.dma_start(part4[:],
                                cc_out[:].rearrange("(g c) k -> c k g", c=64))
            tot = small.tile([64, 2], f32)
            nc.vector.reduce_sum(tot[:], part4[:], axis=AX)

            # ---- moments -> y = std + mean ----
            # tot = sum over 4*_NT equal groups of (mean_g, e2_g)
            mom = small.tile([64, 2], f32)           # [mean, E[x^2]]
            nc.vector.tensor_scalar_mul(mom[:], tot[:], 1.0 / (4 * _NT))
            msq = small.tile([64, 1], f32)
            nc.vector.tensor_mul(msq[:], mom[:, 0:1], mom[:, 0:1])
            var = small.tile([64, 1], f32)
            nc.vector.tensor_sub(var[:], mom[:, 1:2], msq[:])
            std = small.tile([64, 1], f32)
            nc.scalar.activation(std[:], var[:], AF.Sqrt)
            y = small.tile([64, 1], f32)
            nc.vector.tensor_add(y[:], std[:], mom[:, 0:1])

            # ---- MLP: h = relu(w1 @ y + b1); g = sigmoid(w2 @ h + b2) ----
            ph = psum.tile([4, 1], f32)
            nc.tensor.matmul(ph[:], w1t_sb[:], y[:])
            h = small.tile([4, 1], f32)
            nc.scalar.activation(h[:], ph[:], AF.Relu, bias=b1_sb[:, 0:1])
            # w2t is [w2.T | w2.T] so the matmul emits g duplicated over both
            # partition halves, matching the x layout
            pg = psum.tile([128, 1], f32)
            nc.tensor.matmul(pg[:], w2t_sb[:], h[:])
            g = small.tile([128, 1], f32)
            nc.scalar.activation(g[:], pg[:], AF.Sigmoid, bias=b2_sb[:, 0:1])

            # ---- pass 2: out = x * g (in-place DVE mult, stores on ACT ring)
            # mult+store in 4096-wide chunks: stores drain in 2 MiB pieces so
            # they interleave with the next loads instead of 8 MiB bursts
            # (coarse bursts lock the 3-slot ring into load/store oscillation)
            _CH = 4096
            for j in range(_NT):
                t = xp.tile([128, _TILE_N], f32, tag="xtile")
                nc.sync.dma_start(t[:], x[:, j * _TILE_N:(j + 1) * _TILE_N])
                for q in range(_TILE_N // _CH):
                    lo = q * _CH
                    nc.vector.tensor_scalar_mul(t[:, lo:lo + _CH],
                                                t[:, lo:lo + _CH], g[:, 0:1])
                    nc.scalar.dma_start(
                        out[:, j * _TILE_N + lo:j * _TILE_N + lo + _CH],
                        t[:, lo:lo + _CH])

    nc.compile()
    return nc


def kernel(x, w1, b1, w2, b2):
    global _cached_nc, LAST_RESULT
    from concourse.bass_utils import run_bass_kernel_spmd

    x = np.ascontiguousarray(np.asarray(x, dtype=np.float32))
    w1 = np.asarray(w1, dtype=np.float32)
    b1 = np.asarray(b1, dtype=np.float32)
    w2 = np.asarray(w2, dtype=np.float32)
    b2 = np.asarray(b2, dtype=np.float32)

    if _cached_nc is None:
        _cached_nc = _build()
    nc = _cached_nc

    w1t = np.ascontiguousarray(w1.T)                                  # [64, 4]
    b1c = np.ascontiguousarray(b1.reshape(4, 1))
    w2t = np.ascontiguousarray(np.concatenate([w2.T, w2.T], axis=1))  # [4, 128]
    b2c = np.ascontiguousarray(np.concatenate([b2, b2]).reshape(128, 1))

    # x[b, c, d, h, w] -> [b, c, q, n] with q = d//8, n over 8 d-slices
    xv = x.reshape(_B, _C, 4, _FREE)
    in_maps = []
    for i in range(_NCORES):
        b, t = divmod(i, 2)
        xs = np.empty((2, _C, _FREE), np.float32)
        xs[0] = xv[b, :, 2 * t]
        xs[1] = xv[b, :, 2 * t + 1]
        in_maps.append({
            "x": xs.reshape(128, _FREE),
            "w1t": w1t, "b1": b1c, "w2t": w2t, "b2": b2c,
        })

    res = run_bass_kernel_spmd(nc, in_maps, list(range(_NCORES)),
                               trace=TRACE, **TRACE_KWARGS)
    LAST_RESULT = res

    outf = np.empty_like(x)
    ov = outf.reshape(_B, _C, 4, _FREE)
    for i in range(_NCORES):
        b, t = divmod(i, 2)
        r = res.results[i]["out"].reshape(2, _C, _FREE)
        ov[b, :, 2 * t] = r[0]
        ov[b, :, 2 * t + 1] = r[1]
    return outf


# revision 10
# speedup vs baseline: 1.2187x; 1.0607x over previous
"""CCALayer3D kernel for 8 Trainium2 NeuronCores.

reference semantics (x: [4, 64, 32, 128, 128] f32):
    mean/var over (D,H,W) per (B,C); y = std + mean
    h = relu(w1 @ y + b1); g = sigmoid(w2 @ h + b2)
    out = x * g[:, :, None, None, None]

Sharding: core i handles batch b = i//2, D-half t = i%2 (16 of 32 d-slices,
64 MiB per core).  Per-core layout [128, 131072]: partition p = s*64 + c where
s splits the core's 16 d-slices into two groups of 8 (so all 128 partitions
carry DMA traffic).  Pass 1 computes per-partition (mean, var) with bn_stats /
bn_aggr — a single DVE read per element, which keeps the DMA read stream at
~290 GB/s (a second engine reading the tile measurably throttles DMA).  The
per-tile moments are merged, converted to additive (sum-of-means, sum-of-E2)
form, and exchanged between the two cores of a batch with a tiny pair
AllGather; the 64->4->64 MLP runs redundantly on every core.  Pass 2 rescales
x by g in-place on DVE, with mult+store chunked to 4096 columns so stores
interleave with the next tile's loads (8 MiB store bursts lock the 3-slot
ring into a load/store oscillation).  Tile width 16384 keeps DMA descriptors
at 64 KiB (measured 384 GB/s clean read vs 315 at 32 KiB).
"""

import numpy as np

_B, _C = 4, 64
_HW = 128 * 128
_FREE = 8 * _HW            # 131072 free elems per partition
_TILE_N = 16384
_NT = _FREE // _TILE_N     # 8 tiles per pass
_NCORES = 8

# test-harness knobs (the grading harness just calls kernel())
TRACE = False
TRACE_KWARGS = {}
LAST_RESULT = None

_cached_nc = None


def _build():
    import concourse.bacc as bacc
    import concourse.tile as tile
    from concourse import mybir

    nc = bacc.Bacc("TRN2", target_bir_lowering=False, debug=False,
                   num_devices=_NCORES)
    f32 = mybir.dt.float32
    AX = mybir.AxisListType.X
    AF = mybir.ActivationFunctionType

    x = nc.dram_tensor("x", [128, _FREE], f32, kind="ExternalInput")
    out = nc.dram_tensor("out", [128, _FREE], f32, kind="ExternalOutput")
    w1t = nc.dram_tensor("w1t", [64, 4], f32, kind="ExternalInput")
    b1 = nc.dram_tensor("b1", [4, 1], f32, kind="ExternalInput")
    w2t = nc.dram_tensor("w2t", [4, 128], f32, kind="ExternalInput")
    b2 = nc.dram_tensor("b2", [128, 1], f32, kind="ExternalInput")

    with tile.TileContext(nc) as tc:
        with (
            tc.tile_pool(name="xp", bufs=3) as xp,
            tc.tile_pool(name="bnp", bufs=2) as bnp,
            tc.tile_pool(name="small", bufs=1) as small,
            tc.tile_pool(name="psum", bufs=2, space="PSUM") as psum,
            tc.tile_pool(name="dram", bufs=1, space="DRAM") as dram,
        ):
            # MLP weights prefetched up front; overlap with pass 1
            w1t_sb = small.tile([64, 4], f32)
            nc.gpsimd.dma_start(w1t_sb[:], w1t[:])
            b1_sb = small.tile([4, 1], f32)
            nc.gpsimd.dma_start(b1_sb[:], b1[:])
            w2t_sb = small.tile([4, 128], f32)
            nc.gpsimd.dma_start(w2t_sb[:], w2t[:])
            b2_sb = small.tile([128, 1], f32)
            nc.gpsimd.dma_start(b2_sb[:], b2[:])

            # ---- pass 1: per-partition mean/M2 via bn_stats (one DVE read) --
            ngrp = _TILE_N // 512           # bn_stats groups per tile
            mv_cols = small.tile([128, 2, _NT], f32)   # (mean, var) per tile
            for j in range(_NT):
                t = xp.tile([128, _TILE_N], f32, tag="xtile")
                nc.sync.dma_start(t[:], x[:, j * _TILE_N:(j + 1) * _TILE_N])
                bnst = bnp.tile([128, ngrp * 6], f32, tag="bnst")
                for k in range(ngrp):
                    nc.vector.bn_stats(bnst[:, k * 6:(k + 1) * 6],
                                       t[:, k * 512:(k + 1) * 512])
                nc.vector.bn_aggr(mv_cols[:, :, j:j + 1],
                                  bnst[:].rearrange("p (g k) -> p g k", k=6))

            # merge the _NT per-tile (mean, var) pairs (equal counts):
            # mean = avg(mean_j), E2 = avg(var_j + mean_j^2)
            means = mv_cols[:, 0, :]
            varis = mv_cols[:, 1, :]
            msq8 = small.tile([128, _NT], f32)
            nc.vector.tensor_mul(msq8[:], means, means)
            e28 = small.tile([128, _NT], f32)
            nc.vector.tensor_add(e28[:], varis, msq8[:])
            # stats carries (sum(mean_j), sum(e2_j)); scales fold into 1/32
            stats = small.tile([128, 2], f32)
            nc.vector.reduce_sum(stats[:, 0:1], means, axis=AX)
            nc.vector.reduce_sum(stats[:, 1:2], e28[:], axis=AX)

            # ---- pair AllGather of partial stats (cheaper than AllReduce) --
            cc_in = dram.tile([128, 2], f32)
            cc_out = dram.tile([256, 2], f32)
            nc.gpsimd.dma_start(cc_in[:], stats[:])
            nc.gpsimd.collective_compute(
                "AllGather", mybir.AluOpType.bypass,
                replica_groups=[[0, 1], [2, 3], [4, 5], [6, 7]],
                ins=[cc_in.opt()], outs=[cc_out.opt()],
            )
            # cc_out rows: [own s0 | own s1 | peer s0 | peer s1] x 64 channels.
            # Fetch as [c, k, g] and reduce over the 4 groups.
            part4 = small.tile([64, 2, 4], f32)
            nc.gpsimd.dma_start(part4[:],
                                cc_out[:].rearrange("(g c) k -> c k g", c=64))
            tot = small.tile([64, 2], f32)
            nc.vector.reduce_sum(tot[:], part4[:], axis=AX)

            # ---- moments -> y = std + mean ----
            # tot = sum over 4*_NT equal groups of (mean_g, e2_g)
            mom = small.tile([64, 2], f32)           # [mean, E[x^2]]
            nc.vector.tensor_scalar_mul(mom[:], tot[:], 1.0 / (4 * _NT))
            msq = small.tile([64, 1], f32)
            nc.vector.tensor_mul(msq[:], mom[:, 0:1], mom[:, 0:1])
            var = small.tile([64, 1], f32)
            nc.vector.tensor_sub(var[:], mom[:, 1:2], msq[:])
            std = small.tile([64, 1], f32)
            nc.scalar.activation(std[:], var[:], AF.Sqrt)
            y = small.tile([64, 1], f32)
            nc.vector.tensor_add(y[:], std[:], mom[:, 0:1])

            # ---- MLP: h = relu(w1 @ y + b1); g = sigmoid(w2 @ h + b2) ----
            ph = psum.tile([4, 1], f32)
            nc.tensor.matmul(ph[:], w1t_sb[:], y[:])
            h = small.tile([4, 1], f32)
            nc.scalar.activation(h[:], ph[:], AF.Relu, bias=b1_sb[:, 0:1])
            # w2t is [w2.T | w2.T] so the matmul emits g duplicated over both
            # partition halves, matching the x layout
            pg = psum.tile([128, 1], f32)
            nc.tensor.matmul(pg[:], w2t_sb[:], h[:])
            g = small.tile([128, 1], f32)
            nc.scalar.activation(g[:], pg[:], AF.Sigmoid, bias=b2_sb[:, 0:1])

            # ---- pass 2: out = x * g (in-place DVE mult, stores on ACT ring)
            # mult+store in 4096-wide chunks: stores drain in 2 MiB pieces so
            # they interleave with the next loads instead of 8 MiB bursts
            # (coarse bursts lock the 3-slot ring into load/store oscillation)
            _CH = 4096
            for j in range(_NT):
                t = xp.tile([128, _TILE_N], f32, tag="xtile")
                nc.sync.dma_start(t[:], x[:, j * _TILE_N:(j + 1) * _TILE_N])
                for q in range(_TILE_N // _CH):
                    lo = q * _CH
                    nc.vector.tensor_scalar_mul(t[:, lo:lo + _CH],
                                                t[:, lo:lo + _CH], g[:, 0:1])
                    nc.scalar.dma_start(
                        out[:, j * _TILE_N + lo:j * _TILE_N + lo + _CH],
                        t[:, lo:lo + _CH])

    nc.compile()
    return nc


def kernel(x, w1, b1, w2, b2):
    global _cached_nc, LAST_RESULT
    from concourse.bass_utils import run_bass_kernel_spmd

    x = np.ascontiguousarray(np.asarray(x, dtype=np.float32))
    w1 = np.asarray(w1, dtype=np.float32)
    b1 = np.asarray(b1, dtype=np.float32)
    w2 = np.asarray(w2, dtype=np.float32)
    b2 = np.asarray(b2, dtype=np.float32)

    if _cached_nc is None:
        _cached_nc = _build()
    nc = _cached_nc

    w1t = np.ascontiguousarray(w1.T)                                  # [64, 4]
    b1c = np.ascontiguousarray(b1.reshape(4, 1))
    w2t = np.ascontiguousarray(np.concatenate([w2.T, w2.T], axis=1))  # [4, 128]
    b2c = np.ascontiguousarray(np.concatenate([b2, b2]).reshape(128, 1))

    # x[b, c, d, h, w] -> [b, c, q, n] with q = d//8, n over 8 d-slices
    xv = x.reshape(_B, _C, 4, _FREE)
    in_maps = []
    for i in range(_NCORES):
        b, t = divmod(i, 2)
        xs = np.empty((2, _C, _FREE), np.float32)
        xs[0] = xv[b, :, 2 * t]
        xs[1] = xv[b, :, 2 * t + 1]
        in_maps.append({
            "x": xs.reshape(128, _FREE),
            "w1t": w1t, "b1": b1c, "w2t": w2t, "b2": b2c,
        })

    res = run_bass_kernel_spmd(nc, in_maps, list(range(_NCORES)),
                               trace=TRACE, **TRACE_KWARGS)
    LAST_RESULT = res

    outf = np.empty_like(x)
    ov = outf.reshape(_B, _C, 4, _FREE)
    for i in range(_NCORES):
        b, t = divmod(i, 2)
        r = res.results[i]["out"].reshape(2, _C, _FREE)
        ov[b, :, 2 * t] = r[0]
        ov[b, :, 2 * t + 1] = r[1]
    return outf
